# revision 1
# baseline (speedup 1.0000x reference)
"""DRAW-model Trainium2 kernel (8 NeuronCores, data-parallel over batch).

Strategy
--------
Pure data parallelism: 8 cores x 64 local batch, zero collectives (the
10-step LSTM recurrence is latency-critical; on-chip collective floors of
~5-10us/call would dominate).  All weights live SBUF-resident in fp16
(fp16 chosen over bf16 for 8x lower quantization error at identical PE
throughput).  Activations are kept feature-major ("transposed") so the
big encoder matmul streams N=512-wide moving operands at full rate:

  * canvas lives permanently in PSUM (4 banks), accumulated across steps
    by the tensor engine itself (start=False accumulation);
  * pixels are stored in "parity plane" order (q=2*(dy%2)+(dx%2)) so 4 of
    the 9 extract_patches shifts alias the x_hat buffer directly and the
    other 5 are two partition-shifted SBUF->SBUF DMA copies each;
  * padding columns of extract_patches are handled by zeroing the
    corresponding rows of enc_kernel on the host (garbage * 0 = 0);
  * the encoder matmul (K=9728 after folding the duplicated h_dec block)
    runs column-tiled: two concurrent 128x64 tiles (tile_position (0,0)
    and (0,64)) computing the i|f and g|o gate column halves, recovering
    full PE utilization at M=64;
  * attention softmax is reformulated transposed (exp -> ones-matmul
    column sum -> reciprocal -> K=1 broadcast matmul) so nothing needs a
    partition-axis reduction;
  * biases are folded in as extra contraction rows (ones-row stationary x
    bias-row moving operand).
"""

import numpy as np

STEPS = 10
UNITS = 256
BL = 64          # local batch per core
NCORES = 8
IMG = 64

# ---------------------------------------------------------------- host index math
def _pix_order():
    # new pixel index n = q*1024 + j*32 + i  ->  original pixel (2j+pr)*64 + (2i+pc)
    # with q = pr*2 + pc
    out = np.empty(4096, np.int64)
    n = 0
    for pr in range(2):
        for pc in range(2):
            for j in range(32):
                for i in range(32):
                    out[n] = (2 * j + pr) * 64 + (2 * i + pc)
                    n += 1
    return out


PIX = _pix_order()


def fold_enc_kernel(W):
    """Collapse extract_patches into the weight: each patch feature copies one
    pixel of x_hat, so patches @ W[:9216] == x_hat_flat @ A with
    A[p,:] = sum of W rows whose feature reads pixel p.  [4096, 1024]."""
    A = np.zeros((4096, W.shape[1]), np.float32)
    r_idx = np.arange(32)
    for dy in range(3):
        rows = 2 * r_idx + dy
        rv = r_idx[rows < 64]
        for dx in range(3):
            cols = 2 * r_idx + dx
            cv = r_idx[cols < 64]
            pix = (2 * rv[:, None] + dy) * 64 + (2 * cv[None, :] + dx)
            feat = (rv[:, None] * 32 + cv[None, :]) * 9 + (dy * 3 + dx)
            A[pix.ravel()] += W[feat.ravel()]
    return A


# ---------------------------------------------------------------- program builder
def build_program(repeat=None, coltile=False, steps=STEPS, has_bdec=False,
                  carry_state=False):
    """Build + compile the per-core Bass program.  Returns nc."""
    import concourse.bacc as bacc
    import concourse.tile as tile
    import concourse.mybir as mybir

    f16 = mybir.dt.float16
    f32 = mybir.dt.float32
    AF = mybir.ActivationFunctionType

    nc = bacc.Bacc("TRN2", target_bir_lowering=False, debug=False,
                   dynamic_dma_scratch_size=2048)

    def din(name, shape, dt):
        return nc.dram_tensor(name, shape, dt, kind="ExternalInput")

    d_encw = din("encw", [128, 32 * 1024], f16)
    d_recw = din("recw", [128, 5 * 1024], f16)
    d_decw = din("decw", [128, 5 * 1024], f16)
    d_wdec = din("wdec", [128, 2 * 4096], f16)
    d_wenc = din("wenc", [128, 32], f16)
    d_wencT = din("wencT", [16, 256], f16)
    d_benc = din("benc", [16, 1], f32)
    d_iden = din("iden", [64, 64], f16)
    d_onescol = din("onescol", [128, 1], f16)
    d_onesst = din("onesst", [128, 64], f16)
    d_onesrow = din("onesrow", [1, 128], f16)
    d_xm1 = din("xm1", [128, 2048], f16)
    if has_bdec:
        d_bdec = din("bdec", [1, 4096], f16)
    d_out = nc.dram_tensor("canvas", [128, 2048], f32, kind="ExternalOutput")

    with tile.TileContext(nc) as tc:
        # ---------------- static SBUF (raw allocs; tracked via shadow memory)
        s_encw = nc.alloc_sbuf_tensor("s_encw", [128, 32, 1024], f16)
        s_recw = nc.alloc_sbuf_tensor("s_recw", [128, 5, 1024], f16)
        s_decw = nc.alloc_sbuf_tensor("s_decw", [128, 5, 1024], f16)
        s_wdec = nc.alloc_sbuf_tensor("s_wdec", [128, 2, 4096], f16)
        s_wenc = nc.alloc_sbuf_tensor("s_wenc", [128, 2, 16], f16)
        s_wencT = nc.alloc_sbuf_tensor("s_wencT", [16, 256], f16)
        s_benc = nc.alloc_sbuf_tensor("s_benc", [16, 1], f32)
        s_iden = nc.alloc_sbuf_tensor("s_iden", [64, 64], f16)
        s_onescol = nc.alloc_sbuf_tensor("s_onescol", [128, 1], f16)
        s_onesst = nc.alloc_sbuf_tensor("s_onesst", [128, 64], f16)
        s_onesrow = nc.alloc_sbuf_tensor("s_onesrow", [1, 128], f16)
        s_xm1 = nc.alloc_sbuf_tensor("s_xm1", [128, 4, 8, 64], f16)
        s_xhat = nc.alloc_sbuf_tensor("s_xhat", [128, 4, 8, 64], f16)
        s_hencT = nc.alloc_sbuf_tensor("s_hencT", [128, 2, 64], f16)
        s_hdecT = nc.alloc_sbuf_tensor("s_hdecT", [128, 2, 64], f16)
        s_zattnT = nc.alloc_sbuf_tensor("s_zattnT", [128, 2, 64], f16)
        s_cenc = nc.alloc_sbuf_tensor("s_cenc", [64, 256], f32)
        s_cdec = nc.alloc_sbuf_tensor("s_cdec", [64, 256], f32)
        if has_bdec:
            s_bdec = nc.alloc_sbuf_tensor("s_bdec", [1, 4096], f16)

        # ---------------- load weights / constants (outside any repeat loop)
        # small tensors first, then enc weights split into chunks so step-0
        # matmuls can start as soon as their K-tiles have landed
        for dst, src in [
            (s_xm1[:, :, :, :], d_xm1.ap()),
            (s_recw[:, :, :], d_recw.ap()),
            (s_decw[:, :, :], d_decw.ap()), (s_wdec[:, :, :], d_wdec.ap()),
            (s_wenc[:, :, :], d_wenc.ap()), (s_wencT[:, :], d_wencT.ap()),
            (s_benc[:, :], d_benc.ap()), (s_iden[:, :], d_iden.ap()),
            (s_onescol[:, :], d_onescol.ap()), (s_onesst[:, :], d_onesst.ap()),
            (s_onesrow[:, :], d_onesrow.ap()),
        ]:
            nc.sync.dma_start(out=dst, in_=src)
        encw_d_ap = d_encw.ap().rearrange("p (t n) -> p t n", n=1024)
        for g in range(16):
            sl = slice(g * 2, (g + 1) * 2)
            nc.sync.dma_start(out=s_encw[:, sl, :], in_=encw_d_ap[:, sl, :])
        if has_bdec:
            nc.sync.dma_start(out=s_bdec[:, :], in_=d_bdec.ap())

        # ---------------- pools
        import contextlib
        ctx = contextlib.ExitStack()
        work = ctx.enter_context(tc.tile_pool(name="work", bufs=2))
        p_za = ctx.enter_context(tc.tile_pool(name="p_za", bufs=1, space="PSUM"))
        p_zb = ctx.enter_context(tc.tile_pool(name="p_zb", bufs=1, space="PSUM"))
        p_sm = ctx.enter_context(tc.tile_pool(name="p_sm", bufs=2, space="PSUM"))
        p_cv = ctx.enter_context(tc.tile_pool(name="p_cv", bufs=1, space="PSUM"))

        def gates(z_if, tg, so, c_s, hT_dst):
            """LSTM gate math.  z_if: [64,512] AP (i|f cols); tg/so: [64,256]
            APs holding tanh(g) and sigmoid(o) (fp16 or f32).  Updates c_s in
            place, writes transposed fp16 h into hT_dst."""
            t_sif = work.tile([64, 512], f32, tag="t_sif")
            h = work.tile([64, 256], f16, tag="h")
            nc.scalar.activation(t_sif[:, :], z_if[:, 0:512], AF.Sigmoid)
            t_u = work.tile([64, 256], f32, tag="t_u")
            nc.vector.tensor_mul(t_u[:, :], t_sif[:, 0:256], tg)
            nc.vector.tensor_mul(c_s[:, :], t_sif[:, 256:512], c_s[:, :])
            nc.vector.tensor_add(c_s[:, :], c_s[:, :], t_u[:, :])
            t_tc = work.tile([64, 256], f32, tag="t_u")
            nc.scalar.activation(t_tc[:, :], c_s[:, :], AF.Tanh)
            nc.vector.tensor_mul(h[:, :], so, t_tc[:, :])
            for k in range(2):
                pt = p_sm.tile([128, 64], f16, tag="sm")
                nc.tensor.transpose(pt[:, :], h[:, k * 128:(k + 1) * 128],
                                    s_iden[:, :])
                nc.scalar.activation(hT_dst[:, k, :], pt[:, :], AF.Copy)

        def body():
            if not carry_state:
                nc.vector.memset(s_cenc[:, :], 0.0)
                nc.vector.memset(s_cdec[:, :], 0.0)
            canvas = p_cv.tile([128, 32, 64], f32, tag="canvas")

            for t in range(steps):
                # ---- x_hat = (x-1) + sigmoid(-canvas)  [fp16, parity planes]
                # per plane: plane q reads only PSUM bank q, so each plane's
                # sigmoid can start as soon as that bank's canvas matmuls land
                for q in range(4):
                    xh = s_xhat[:, q, :, :]
                    if t == 0:
                        nc.scalar.activation(xh, s_xm1[:, q, :, :], AF.Copy,
                                             bias=0.5)
                    else:
                        nc.scalar.activation(xh, canvas[:, 8 * q:8 * (q + 1), :],
                                             AF.Sigmoid, scale=-1.0)
                        nc.vector.tensor_add(xh, xh, s_xm1[:, q, :, :])

                # ---- encoder matmul (patch extraction folded into weights:
                # the 32 x_hat plane chunks ARE the stationary K-tiles)
                psA = p_za.tile([64, 512], f32, tag="za")
                psB = p_zb.tile([128, 512], f32, tag="zb")
                stat = []   # (stationary AP, rhs buffer handle, rhs tile idx)
                for q in range(4):
                    for m in range(8):
                        stat.append((s_xhat[:, q, m, :], s_encw, q * 8 + m))
                if t > 0:
                    stat.append((s_hencT[:, 0, :], s_recw, 0))
                    stat.append((s_hencT[:, 1, :], s_recw, 1))
                    stat.append((s_hdecT[:, 0, :], s_recw, 2))
                    stat.append((s_hdecT[:, 1, :], s_recw, 3))
                stat.append((s_onesst[:, :], s_recw, 4))
                last = len(stat) - 1
                bhalf = psB[64:128, :] if coltile else psB[0:64, :]
                for j, (st, buf, jj) in enumerate(stat):
                    nc.tensor.matmul(
                        psA[:, :], st, buf[:, jj, 0:512],
                        start=(j == 0), stop=(j == last),
                        tile_position=(0, 0) if coltile else None,
                        skip_group_check=True)
                    nc.tensor.matmul(
                        bhalf, st, buf[:, jj, 512:1024],
                        start=(j == 0), stop=(j == last),
                        tile_position=(0, 64) if coltile else None,
                        skip_group_check=True)
                if coltile:
                    # tanh(g)/sigmoid(o) on partitions 64-127, then DMA-shift
                    # the fp16 result down to partitions 0-63
                    go_hi = work.tile([128, 2, 256], f16, tag="go_hi")
                    go_lo = work.tile([128, 2, 256], f16, tag="go_lo")
                    nc.scalar.activation(go_hi[64:128, 0, :], psB[64:128, 0:256],
                                         AF.Tanh)
                    nc.scalar.activation(go_hi[64:128, 1, :], psB[64:128, 256:512],
                                         AF.Sigmoid)
                    nc.sync.dma_start(out=go_lo[0:64, :, :],
                                      in_=go_hi[64:128, :, :])
                    tg, so = go_lo[0:64, 0, :], go_lo[0:64, 1, :]
                else:
                    t_tg = work.tile([64, 256], f32, tag="t_tg")
                    t_so = work.tile([64, 256], f32, tag="t_so")
                    nc.scalar.activation(t_tg[:, :], psB[0:64, 0:256], AF.Tanh)
                    nc.scalar.activation(t_so[:, :], psB[0:64, 256:512],
                                         AF.Sigmoid)
                    tg, so = t_tg[:, :], t_so[:, :]
                gates(psA[:, :], tg, so, s_cenc, s_hencT)

                # ---- attention (transposed, softmax via ones-matmul)
                ps_log = p_sm.tile([16, 64], f32, tag="sm")
                nc.tensor.matmul(ps_log[0:10, :], s_wenc[:, 0, 0:10],
                                 s_hencT[:, 0, :], start=True, stop=False,
                                 skip_group_check=True)
                nc.tensor.matmul(ps_log[0:10, :], s_wenc[:, 1, 0:10],
                                 s_hencT[:, 1, :], start=False, stop=True,
                                 skip_group_check=True)
                expT = work.tile([16, 64], f16, tag="expT")
                nc.scalar.activation(expT[0:10, :], ps_log[0:10, :], AF.Exp,
                                     bias=s_benc[0:10, 0:1])
                ps_cs = p_sm.tile([16, 64], f32, tag="sm")
                nc.tensor.matmul(ps_cs[0:1, :], s_onescol[0:10, 0:1],
                                 expT[0:10, :], start=True, stop=True,
                                 skip_group_check=True)
                rec32 = work.tile([1, 64], f32, tag="rec32")
                nc.vector.reciprocal(rec32[:, :], ps_cs[0:1, :])
                rec16 = work.tile([1, 64], f16, tag="rec16")
                nc.scalar.activation(rec16[:, :], rec32[:, :], AF.Copy)
                ps_bc = p_sm.tile([128, 64], f32, tag="sm")
                nc.tensor.matmul(ps_bc[:, :], s_onesrow[0:1, 0:128],
                                 rec16[0:1, :], start=True, stop=True,
                                 skip_group_check=True)
                bc = work.tile([128, 64], f32, tag="bc")
                nc.scalar.activation(bc[:, :], ps_bc[:, :], AF.Copy)
                for k in range(2):
                    ps_zat = p_sm.tile([128, 64], f32, tag="sm")
                    nc.tensor.matmul(ps_zat[:, :],
                                     s_wencT[0:10, k * 128:(k + 1) * 128],
                                     expT[0:10, :], start=True, stop=True,
                                     skip_group_check=True)
                    nc.vector.tensor_mul(s_zattnT[:, k, :], ps_zat[:, :],
                                         bc[:, :])

                # ---- decoder LSTM matmul (plain M=64, both col halves at p0-63)
                psA2 = p_za.tile([64, 512], f32, tag="za")
                psB2 = p_zb.tile([128, 512], f32, tag="zb")
                dstat = [(s_zattnT[:, 0, :], 0), (s_zattnT[:, 1, :], 1)]
                if t > 0:
                    dstat += [(s_hdecT[:, 0, :], 2), (s_hdecT[:, 1, :], 3)]
                dstat.append((s_onesst[:, :], 4))
                dlast = len(dstat) - 1
                for j, (st, jj) in enumerate(dstat):
                    nc.tensor.matmul(psA2[:, :], st, s_decw[:, jj, 0:512],
                                     start=(j == 0), stop=(j == dlast),
                                     skip_group_check=True)
                    nc.tensor.matmul(psB2[0:64, :], st, s_decw[:, jj, 512:1024],
                                     start=(j == 0), stop=(j == dlast),
                                     skip_group_check=True)
                t_tg = work.tile([64, 256], f32, tag="t_tg")
                t_so = work.tile([64, 256], f32, tag="t_so")
                nc.scalar.activation(t_tg[:, :], psB2[0:64, 0:256], AF.Tanh)
                nc.scalar.activation(t_so[:, :], psB2[0:64, 256:512], AF.Sigmoid)
                gates(psA2[:, :], t_tg[:, :], t_so[:, :], s_cdec, s_hdecT)

                # ---- canvas += W_dec^T @ h_dec  (PSUM-resident accumulation)
                # plane-major so bank q completes early and the next step's
                # sigmoid(plane q) / patch copies can overlap banks q+1..3
                for q in range(4):
                    for k in range(2):
                        for m in range(8 * q, 8 * (q + 1)):
                            # start=True clears the whole PSUM bank (8 chunks):
                            # only the first matmul touching a bank may set it
                            nc.tensor.matmul(
                                canvas[:, m, :],
                                s_wdec[:, k, m * 128:(m + 1) * 128],
                                s_hdecT[:, k, :],
                                start=(t == 0 and k == 0 and m % 8 == 0),
                                stop=(t == steps - 1 and k == 1
                                      and not has_bdec),
                                skip_group_check=True)
                if has_bdec:
                    for m in range(32):
                        nc.tensor.matmul(
                            canvas[:, m, :],
                            s_bdec[0:1, m * 128:(m + 1) * 128],
                            s_onesrow[0:1, 0:64],
                            start=False,
                            stop=(t == steps - 1 and m == 31),
                            skip_group_check=True)

            # evacuate canvas PSUM -> SBUF -> DRAM in 8 chunks
            for m4 in range(8):
                cv = work.tile([128, 256], f32, tag="cvout")
                nc.scalar.activation(cv[:, :], canvas[:, m4 * 4:(m4 + 1) * 4, :],
                                     AF.Copy)
                nc.sync.dma_start(
                    out=d_out.ap()[:, m4 * 256:(m4 + 1) * 256], in_=cv[:, :])

        if carry_state:
            nc.vector.memset(s_cenc[:, :], 0.0)
            nc.vector.memset(s_cdec[:, :], 0.0)
        if repeat:
            with tc.For_i(0, repeat, 1):
                body()
        else:
            body()
        ctx.close()

    nc.compile()
    return nc


# ---------------------------------------------------------------- host packing
def host_pack(inputs):
    """Preprocess full inputs -> (shared weight map, per-core input maps)."""
    f16 = np.float16
    ek = np.asarray(inputs["enc_kernel"], np.float32)
    A = fold_enc_kernel(ek[:9216])[PIX]       # [4096, 1024] parity-plane order
    hdf = ek[9216:9472] + ek[9472:9728]
    enc_rec = np.asarray(inputs["enc_rec"], np.float32)
    enc_bias = np.asarray(inputs["enc_bias"], np.float32)
    dec_k = np.asarray(inputs["dec_kernel"], np.float32)
    dec_rec = np.asarray(inputs["dec_rec"], np.float32)
    dec_bias = np.asarray(inputs["dec_bias"], np.float32)
    W_enc = np.asarray(inputs["W_enc"], np.float32)
    b_enc = np.asarray(inputs["b_enc"], np.float32)
    W_dec = np.asarray(inputs["W_dec"], np.float32)
    b_dec = np.asarray(inputs["b_dec"], np.float32)

    encw = A.reshape(32, 128, 1024).transpose(1, 0, 2).reshape(128, -1)

    def brow(bias):
        t = np.zeros((128, 1024), np.float32)
        t[0] = bias
        return t

    recw = np.stack([enc_rec[0:128], enc_rec[128:256], hdf[0:128], hdf[128:256],
                     brow(enc_bias)]).transpose(1, 0, 2).reshape(128, -1)
    decw = np.stack([dec_k[0:128], dec_k[128:256], dec_rec[0:128],
                     dec_rec[128:256], brow(dec_bias)]
                    ).transpose(1, 0, 2).reshape(128, -1)
    wdec = W_dec[:, PIX].reshape(2, 128, 4096).transpose(1, 0, 2).reshape(128, -1)
    wenc = np.zeros((128, 2, 16), np.float32)
    wenc[:, 0, 0:10] = W_enc[0:128]
    wenc[:, 1, 0:10] = W_enc[128:256]
    wencT = np.zeros((16, 256), np.float32)
    wencT[0:10] = W_enc.T
    benc = np.zeros((16, 1), np.float32)
    benc[0:10, 0] = b_enc
    onesst = np.zeros((128, 64), np.float32)
    onesst[0] = 1.0

    shared = {
        "encw": encw.astype(f16), "recw": recw.astype(f16),
        "decw": decw.astype(f16), "wdec": wdec.astype(f16),
        "wenc": wenc.reshape(128, 32).astype(f16),
        "wencT": wencT.astype(f16), "benc": benc,
        "iden": np.eye(64, dtype=f16),
        "onescol": np.ones((128, 1), f16),
        "onesst": onesst.astype(f16),
        "onesrow": np.ones((1, 128), f16),
    }
    has_bdec = bool(np.any(b_dec))
    if has_bdec:
        shared["bdec"] = b_dec[PIX].reshape(1, 4096).astype(f16)

    x = np.asarray(inputs["x"], np.float32)
    B = x.shape[0]
    assert B == NCORES * BL, f"expected batch {NCORES * BL}, got {B}"
    in_maps = []
    for c in range(NCORES):
        xc = x[c * BL:(c + 1) * BL].reshape(BL, 4096).T[PIX] - 1.0
        xm1 = (xc.reshape(4, 8, 128, BL).transpose(2, 0, 1, 3)
               .reshape(128, 2048).astype(f16))
        m = dict(shared)
        m["xm1"] = xm1
        in_maps.append(m)
    return in_maps, has_bdec


def unpack_output(res_core):
    """[128, 2048] f32 parity-major canvas^T -> [BL, 64, 64]."""
    buf = res_core.reshape(128, 32, 64).transpose(1, 0, 2).reshape(4096, BL)
    out = np.empty((4096, BL), np.float32)
    out[PIX] = buf
    return out.T.reshape(BL, IMG, IMG)


_NC_CACHE = {}


def _get_nc(repeat=None, coltile=True, has_bdec=False):
    key = (repeat, coltile, has_bdec)
    if key not in _NC_CACHE:
        _NC_CACHE[key] = build_program(repeat=repeat, coltile=coltile,
                                       has_bdec=has_bdec)
    return _NC_CACHE[key]


def kernel(**inputs):
    import sys
    if "/opt/trn_rl_repo" not in sys.path:
        sys.path.insert(0, "/opt/trn_rl_repo")
    from concourse import bass_utils

    in_maps, has_bdec = host_pack(inputs)
    nc = _get_nc(has_bdec=has_bdec)
    res = bass_utils.run_bass_kernel_spmd(nc, in_maps,
                                          core_ids=list(range(NCORES)))
    outs = [unpack_output(np.asarray(res.results[c]["canvas"]))
            for c in range(NCORES)]
    return np.concatenate(outs, axis=0).astype(
        np.asarray(inputs["x"]).dtype, copy=False)



# revision 6
# speedup vs baseline: 1.2747x; 1.2747x over previous
"""DRAW-model Trainium2 kernel (8 NeuronCores, data-parallel over batch).

Strategy (v2)
-------------
Pure data parallelism: 8 cores x 64 local batch, zero collectives.  All
weights SBUF-resident in fp16.  Activations feature-major ("transposed")
so the big encoder matmul streams N=512-wide moving operands at full PE
rate.  Key structural choices:

  * canvas lives permanently in PSUM (4 banks), accumulated across steps
    by the tensor engine (start=False accumulation);
  * pixels in "parity plane" order; extract_patches folded into the
    encoder weight (host side);
  * FEATURE-SPLIT column tiling: both LSTM matmuls run two concurrent
    64-col PE groups, where group A computes all four gates for features
    0-127 (PSUM partitions 0-63) and group B for features 128-255
    (partitions 64-127).  The LSTM cell math is elementwise per feature,
    so both halves proceed with no cross-partition traffic (v1 needed a
    1.9us SBUF->SBUF DMA partition shift per step);
  * all sigmoids become tanh via sigmoid(x) = 0.5*tanh(0.5x)+0.5 with
    the 0.5 input scales folded into the weights, so every activation
    (tanh, exp, copy) lives in ONE ACT table set -> zero per-step
    ACT_TABLE_LOADs (v1 paid 2x 1.28us per step);
  * x_hat = x - sigmoid(canvas) is tracked as x_hat_alt = 2*x_hat =
    (2x-1) + tanh(-canvas/2), with the compensating 0.5 folded into the
    encoder weight pixel rows;
  * attention softmax reformulated transposed (exp -> ones-matmul column
    sum -> reciprocal -> K=1 broadcast matmul);
  * biases folded in as extra contraction rows.
"""

import numpy as np

STEPS = 10
UNITS = 256
BL = 64          # local batch per core
NCORES = 8
IMG = 64

# ---------------------------------------------------------------- host index math
def _pix_order():
    # new pixel index n = q*1024 + j*32 + i  ->  original pixel (2j+pr)*64 + (2i+pc)
    # with q = pr*2 + pc
    out = np.empty(4096, np.int64)
    n = 0
    for pr in range(2):
        for pc in range(2):
            for j in range(32):
                for i in range(32):
                    out[n] = (2 * j + pr) * 64 + (2 * i + pc)
                    n += 1
    return out


PIX = _pix_order()


def fold_enc_kernel(W):
    """Collapse extract_patches into the weight: each patch feature copies one
    pixel of x_hat, so patches @ W[:9216] == x_hat_flat @ A with
    A[p,:] = sum of W rows whose feature reads pixel p.  [4096, 1024]."""
    A = np.zeros((4096, W.shape[1]), np.float32)
    r_idx = np.arange(32)
    for dy in range(3):
        rows = 2 * r_idx + dy
        rv = r_idx[rows < 64]
        for dx in range(3):
            cols = 2 * r_idx + dx
            cv = r_idx[cols < 64]
            pix = (2 * rv[:, None] + dy) * 64 + (2 * cv[None, :] + dx)
            feat = (rv[:, None] * 32 + cv[None, :]) * 9 + (dy * 3 + dx)
            A[pix.ravel()] += W[feat.ravel()]
    return A


def repack_gate_cols(W):
    """[rows, 1024] with keras layout [i|f|g|o] x 256 features ->
    feature-split layout: cols [h*512 + g*128 + j] = old [g*256 + h*128 + j].
    Also folds the sigmoid-as-tanh input scale 0.5 into the i, f, o gates."""
    out = np.empty_like(W)
    for h in range(2):
        for g in range(4):
            s = 0.5 if g != 2 else 1.0
            out[:, h * 512 + g * 128: h * 512 + (g + 1) * 128] = (
                W[:, g * 256 + h * 128: g * 256 + (h + 1) * 128] * s)
    return out


# ---------------------------------------------------------------- program builder
def build_program(repeat=None, steps=STEPS, has_bdec=False, carry_state=False):
    """Build + compile the per-core Bass program.  Returns nc."""
    import concourse.bacc as bacc
    import concourse.tile as tile
    import concourse.mybir as mybir

    f16 = mybir.dt.float16
    f32 = mybir.dt.float32
    AF = mybir.ActivationFunctionType
    ALU = mybir.AluOpType

    nc = bacc.Bacc("TRN2", target_bir_lowering=False, debug=False,
                   dynamic_dma_scratch_size=2048)

    def din(name, shape, dt):
        return nc.dram_tensor(name, shape, dt, kind="ExternalInput")

    d_encw = din("encw", [128, 32 * 1024], f16)
    d_recw = din("recw", [128, 5 * 1024], f16)
    d_decw = din("decw", [128, 5 * 1024], f16)
    d_wdec = din("wdec", [128, 2 * 4096], f16)
    d_wenc = din("wenc", [128, 32], f16)
    d_wencT = din("wencT", [16, 256], f16)
    d_benc = din("benc", [16, 1], f32)
    d_iden = din("iden", [128, 128], f16)
    d_onescol = din("onescol", [128, 1], f16)
    d_onesst = din("onesst", [128, 64], f16)
    d_onesrow = din("onesrow", [1, 128], f16)
    d_xm1 = din("xm1", [128, 2048], f16)
    if has_bdec:
        d_bdec = din("bdec", [1, 4096], f16)
    d_out = nc.dram_tensor("canvas", [128, 2048], f32, kind="ExternalOutput")

    with tile.TileContext(nc) as tc:
        # ---------------- static SBUF (raw allocs; tracked via shadow memory)
        s_encw = nc.alloc_sbuf_tensor("s_encw", [128, 32, 1024], f16)
        s_recw = nc.alloc_sbuf_tensor("s_recw", [128, 5, 1024], f16)
        s_decw = nc.alloc_sbuf_tensor("s_decw", [128, 5, 1024], f16)
        s_wdec = nc.alloc_sbuf_tensor("s_wdec", [128, 2, 4096], f16)
        s_wenc = nc.alloc_sbuf_tensor("s_wenc", [128, 2, 16], f16)
        s_wencT = nc.alloc_sbuf_tensor("s_wencT", [16, 256], f16)
        s_benc = nc.alloc_sbuf_tensor("s_benc", [16, 1], f32)
        s_iden = nc.alloc_sbuf_tensor("s_iden", [128, 128], f16)
        s_onescol = nc.alloc_sbuf_tensor("s_onescol", [128, 1], f16)
        s_onesst = nc.alloc_sbuf_tensor("s_onesst", [128, 64], f16)
        s_onesrow = nc.alloc_sbuf_tensor("s_onesrow", [1, 128], f16)
        s_xm1 = nc.alloc_sbuf_tensor("s_xm1", [128, 4, 8, 64], f16)
        s_xhat = nc.alloc_sbuf_tensor("s_xhat", [128, 4, 8, 64], f16)
        s_hencT = nc.alloc_sbuf_tensor("s_hencT", [128, 2, 64], f16)
        s_hdecT = nc.alloc_sbuf_tensor("s_hdecT", [128, 2, 64], f16)
        s_zattnT = nc.alloc_sbuf_tensor("s_zattnT", [128, 2, 64], f16)
        s_cenc = nc.alloc_sbuf_tensor("s_cenc", [128, 128], f32)
        s_cdec = nc.alloc_sbuf_tensor("s_cdec", [128, 128], f32)
        if has_bdec:
            s_bdec = nc.alloc_sbuf_tensor("s_bdec", [1, 4096], f16)

        # ---------------- load weights / constants (outside any repeat loop)
        for dst, src in [
            (s_xm1[:, :, :, :], d_xm1.ap()),
            (s_recw[:, :, :], d_recw.ap()),
            (s_decw[:, :, :], d_decw.ap()), (s_wdec[:, :, :], d_wdec.ap()),
            (s_wenc[:, :, :], d_wenc.ap()), (s_wencT[:, :], d_wencT.ap()),
            (s_benc[:, :], d_benc.ap()), (s_iden[:, :], d_iden.ap()),
            (s_onescol[:, :], d_onescol.ap()), (s_onesst[:, :], d_onesst.ap()),
            (s_onesrow[:, :], d_onesrow.ap()),
        ]:
            nc.sync.dma_start(out=dst, in_=src)
        encw_d_ap = d_encw.ap().rearrange("p (t n) -> p t n", n=1024)
        for g in range(16):
            sl = slice(g * 2, (g + 1) * 2)
            nc.sync.dma_start(out=s_encw[:, sl, :], in_=encw_d_ap[:, sl, :])
        if has_bdec:
            nc.sync.dma_start(out=s_bdec[:, :], in_=d_bdec.ap())

        # ---------------- pools
        import contextlib
        ctx = contextlib.ExitStack()
        work = ctx.enter_context(tc.tile_pool(name="work", bufs=2))
        p_z = ctx.enter_context(tc.tile_pool(name="p_z", bufs=1, space="PSUM"))
        p_sm = ctx.enter_context(tc.tile_pool(name="p_sm", bufs=3, space="PSUM"))
        p_cv = ctx.enter_context(tc.tile_pool(name="p_cv", bufs=1, space="PSUM"))

        def gates(psZ, c_s, hT_dst):
            """LSTM gate math on the feature-split layout.  psZ: [128, 512]
            PSUM AP with per-partition-half columns [i|f|g|o] x 128 features
            (i, f, o pre-scaled by 0.5 in the weights).  c_s: [128, 128] f32
            state, updated in place.  hT_dst: [128, 128] view of the
            transposed-h fp16 destination."""
            t = work.tile([128, 512], f32, tag="tz")
            nc.scalar.activation(t[:, :], psZ[:, 0:512], AF.Tanh)
            sf = work.tile([128, 128], f32, tag="sf")
            nc.vector.tensor_scalar(sf[:, :], t[:, 128:256], 0.5, 0.5,
                                    ALU.mult, ALU.add)
            nc.vector.tensor_mul(c_s[:, :], sf[:, :], c_s[:, :])
            si = work.tile([128, 128], f32, tag="si")
            nc.vector.tensor_scalar(si[:, :], t[:, 0:128], 0.5, 0.5,
                                    ALU.mult, ALU.add)
            u = work.tile([128, 128], f32, tag="u")
            nc.vector.tensor_mul(u[:, :], si[:, :], t[:, 256:384])
            nc.vector.tensor_add(c_s[:, :], c_s[:, :], u[:, :])
            tc_ = work.tile([128, 128], f32, tag="tc")
            nc.scalar.activation(tc_[:, :], c_s[:, :], AF.Tanh)
            so = work.tile([128, 128], f32, tag="so")
            nc.vector.tensor_scalar(so[:, :], t[:, 384:512], 0.5, 0.5,
                                    ALU.mult, ALU.add)
            h = work.tile([128, 128], f16, tag="h")
            nc.vector.tensor_mul(h[:, :], so[:, :], tc_[:, :])
            pt = p_sm.tile([128, 128], f16, tag="sm")
            nc.tensor.transpose(pt[:, :], h[:, :], s_iden[:, :])
            nc.vector.tensor_scalar_mul(hT_dst, pt[:, :], 1.0)

        def body():
            if not carry_state:
                nc.vector.memset(s_cenc[:, :], 0.0)
                nc.vector.memset(s_cdec[:, :], 0.0)
            canvas = p_cv.tile([128, 32, 64], f32, tag="canvas")

            for t in range(steps):
                # ---- encoder matmul; group A -> psZ[0:64] (features 0-127),
                # group B -> psZ[64:128] (features 128-255)
                psZ = p_z.tile([128, 512], f32, tag="z")
                stat = []   # (stationary AP, rhs buffer handle, rhs tile idx)
                if t > 0:
                    stat.append((s_hencT[:, 0, :], s_recw, 0))
                    stat.append((s_hencT[:, 1, :], s_recw, 1))
                    stat.append((s_hdecT[:, 0, :], s_recw, 2))
                    stat.append((s_hdecT[:, 1, :], s_recw, 3))
                stat.append((s_onesst[:, :], s_recw, 4))
                xh = s_xm1 if t == 0 else s_xhat
                for q in range(4):
                    for m in range(8):
                        stat.append((xh[:, q, m, :], s_encw, q * 8 + m))
                last = len(stat) - 1
                for j, (st, buf, jj) in enumerate(stat):
                    nc.tensor.matmul(
                        psZ[0:64, :], st, buf[:, jj, 0:512],
                        start=(j == 0), stop=(j == last),
                        tile_position=(0, 0), skip_group_check=True)
                    nc.tensor.matmul(
                        psZ[64:128, :], st, buf[:, jj, 512:1024],
                        start=(j == 0), stop=(j == last),
                        tile_position=(0, 64), skip_group_check=True)
                gates(psZ[:, :], s_cenc, s_hencT[:, :, :])

                # ---- attention (transposed, softmax via ones-matmul)
                ps_log = p_sm.tile([16, 64], f32, tag="sm")
                nc.tensor.matmul(ps_log[0:10, :], s_wenc[:, 0, 0:10],
                                 s_hencT[:, 0, :], start=True, stop=False,
                                 skip_group_check=True)
                nc.tensor.matmul(ps_log[0:10, :], s_wenc[:, 1, 0:10],
                                 s_hencT[:, 1, :], start=False, stop=True,
                                 skip_group_check=True)
                expT = work.tile([16, 64], f16, tag="expT")
                nc.scalar.activation(expT[0:10, :], ps_log[0:10, :], AF.Exp,
                                     bias=s_benc[0:10, 0:1])
                ps_cs = p_sm.tile([16, 64], f32, tag="sm")
                nc.tensor.matmul(ps_cs[0:1, :], s_onescol[0:10, 0:1],
                                 expT[0:10, :], start=True, stop=True,
                                 skip_group_check=True)
                rec32 = work.tile([1, 64], f32, tag="rec32")
                nc.vector.reciprocal(rec32[:, :], ps_cs[0:1, :])
                rec16 = work.tile([1, 64], f16, tag="rec16")
                nc.vector.tensor_scalar_mul(rec16[:, :], rec32[:, :], 1.0)
                ps_bc = p_sm.tile([128, 64], f32, tag="sm")
                nc.tensor.matmul(ps_bc[:, :], s_onesrow[0:1, 0:128],
                                 rec16[0:1, :], start=True, stop=True,
                                 skip_group_check=True)
                bc = work.tile([128, 64], f32, tag="bcs")
                nc.scalar.activation(bc[:, :], ps_bc[:, :], AF.Copy)
                for k in range(2):
                    ps_zat = p_sm.tile([128, 64], f32, tag="sm")
                    nc.tensor.matmul(ps_zat[:, :],
                                     s_wencT[0:10, k * 128:(k + 1) * 128],
                                     expT[0:10, :], start=True, stop=True,
                                     skip_group_check=True)
                    nc.vector.tensor_mul(s_zattnT[:, k, :], ps_zat[:, :],
                                         bc[:, :])

                # ---- decoder LSTM matmul (feature-split col-tiled like enc)
                psZ2 = p_z.tile([128, 512], f32, tag="z")
                dstat = [(s_zattnT[:, 0, :], 0), (s_zattnT[:, 1, :], 1)]
                if t > 0:
                    dstat += [(s_hdecT[:, 0, :], 2), (s_hdecT[:, 1, :], 3)]
                dstat.append((s_onesst[:, :], 4))
                dlast = len(dstat) - 1
                for j, (st, jj) in enumerate(dstat):
                    nc.tensor.matmul(psZ2[0:64, :], st, s_decw[:, jj, 0:512],
                                     start=(j == 0), stop=(j == dlast),
                                     tile_position=(0, 0),
                                     skip_group_check=True)
                    nc.tensor.matmul(psZ2[64:128, :], st,
                                     s_decw[:, jj, 512:1024],
                                     start=(j == 0), stop=(j == dlast),
                                     tile_position=(0, 64),
                                     skip_group_check=True)
                gates(psZ2[:, :], s_cdec, s_hdecT[:, :, :])

                # ---- canvas += W_dec^T @ h_dec  (PSUM-resident accumulation)
                # plane-major so bank q completes early and the next step's
                # tanh(plane q) / enc chunk matmuls can overlap banks q+1..3
                for q in range(4):
                    for k in range(2):
                        for m in range(8 * q, 8 * (q + 1)):
                            nc.tensor.matmul(
                                canvas[:, m, :],
                                s_wdec[:, k, m * 128:(m + 1) * 128],
                                s_hdecT[:, k, :],
                                start=(t == 0 and k == 0 and m % 8 == 0),
                                stop=(t == steps - 1 and k == 1
                                      and not has_bdec),
                                skip_group_check=True)
                if has_bdec:
                    for m in range(32):
                        nc.tensor.matmul(
                            canvas[:, m, :],
                            s_bdec[0:1, m * 128:(m + 1) * 128],
                            s_onesrow[0:1, 0:64],
                            start=False,
                            stop=(t == steps - 1 and m == 31),
                            skip_group_check=True)

                # ---- x_hat_alt for the next step: (2x-1) + tanh(-canvas/2)
                if t < steps - 1:
                    for q in range(4):
                        xq = s_xhat[:, q, :, :]
                        nc.scalar.activation(xq, canvas[:, 8 * q:8 * (q + 1), :],
                                             AF.Tanh, scale=-0.5)
                        nc.vector.tensor_add(xq, xq, s_xm1[:, q, :, :])

            # evacuate canvas PSUM -> SBUF -> DRAM per bank (ACT/DVE alternate)
            for q in range(4):
                cv = work.tile([128, 512], f32, tag="cvout")
                if q % 2 == 0:
                    nc.scalar.activation(cv[:, :],
                                         canvas[:, 8 * q:8 * (q + 1), :],
                                         AF.Copy)
                else:
                    nc.vector.tensor_scalar_mul(
                        cv[:, :], canvas[:, 8 * q:8 * (q + 1), :], 1.0)
                nc.sync.dma_start(
                    out=d_out.ap()[:, q * 512:(q + 1) * 512], in_=cv[:, :])

        if carry_state:
            nc.vector.memset(s_cenc[:, :], 0.0)
            nc.vector.memset(s_cdec[:, :], 0.0)
        if repeat:
            with tc.For_i(0, repeat, 1):
                body()
        else:
            body()
        ctx.close()

    nc.compile()
    return nc


# ---------------------------------------------------------------- host packing
def host_pack(inputs):
    """Preprocess full inputs -> (per-core input maps, has_bdec)."""
    f16 = np.float16
    ek = np.asarray(inputs["enc_kernel"], np.float32)
    # pixel rows scaled by 0.5: the kernel feeds x_hat_alt = 2*x_hat
    A = repack_gate_cols(fold_enc_kernel(ek[:9216])[PIX] * 0.5)
    hdf = repack_gate_cols(ek[9216:9472] + ek[9472:9728])
    enc_rec = repack_gate_cols(np.asarray(inputs["enc_rec"], np.float32))
    enc_bias = np.asarray(inputs["enc_bias"], np.float32)
    dec_k = repack_gate_cols(np.asarray(inputs["dec_kernel"], np.float32))
    dec_rec = repack_gate_cols(np.asarray(inputs["dec_rec"], np.float32))
    dec_bias = np.asarray(inputs["dec_bias"], np.float32)
    W_enc = np.asarray(inputs["W_enc"], np.float32)
    b_enc = np.asarray(inputs["b_enc"], np.float32)
    W_dec = np.asarray(inputs["W_dec"], np.float32)
    b_dec = np.asarray(inputs["b_dec"], np.float32)

    encw = A.reshape(32, 128, 1024).transpose(1, 0, 2).reshape(128, -1)

    def brow(bias1024):
        t = np.zeros((128, 1024), np.float32)
        t[0] = bias1024
        return t

    eb = repack_gate_cols(enc_bias[None, :])[0]
    db = repack_gate_cols(dec_bias[None, :])[0]
    recw = np.stack([enc_rec[0:128], enc_rec[128:256], hdf[0:128], hdf[128:256],
                     brow(eb)]).transpose(1, 0, 2).reshape(128, -1)
    decw = np.stack([dec_k[0:128], dec_k[128:256], dec_rec[0:128],
                     dec_rec[128:256], brow(db)]
                    ).transpose(1, 0, 2).reshape(128, -1)
    wdec = W_dec[:, PIX].reshape(2, 128, 4096).transpose(1, 0, 2).reshape(128, -1)
    wenc = np.zeros((128, 2, 16), np.float32)
    wenc[:, 0, 0:10] = W_enc[0:128]
    wenc[:, 1, 0:10] = W_enc[128:256]
    wencT = np.zeros((16, 256), np.float32)
    wencT[0:10] = W_enc.T
    benc = np.zeros((16, 1), np.float32)
    benc[0:10, 0] = b_enc
    onesst = np.zeros((128, 64), np.float32)
    onesst[0] = 1.0

    shared = {
        "encw": encw.astype(f16), "recw": recw.astype(f16),
        "decw": decw.astype(f16), "wdec": wdec.astype(f16),
        "wenc": wenc.reshape(128, 32).astype(f16),
        "wencT": wencT.astype(f16), "benc": benc,
        "iden": np.eye(128, dtype=f16),
        "onescol": np.ones((128, 1), f16),
        "onesst": onesst.astype(f16),
        "onesrow": np.ones((1, 128), f16),
    }
    has_bdec = bool(np.any(b_dec))
    if has_bdec:
        shared["bdec"] = b_dec[PIX].reshape(1, 4096).astype(f16)

    x = np.asarray(inputs["x"], np.float32)
    B = x.shape[0]
    assert B == NCORES * BL, f"expected batch {NCORES * BL}, got {B}"
    in_maps = []
    for c in range(NCORES):
        xc = 2.0 * x[c * BL:(c + 1) * BL].reshape(BL, 4096).T[PIX] - 1.0
        xm1 = (xc.reshape(4, 8, 128, BL).transpose(2, 0, 1, 3)
               .reshape(128, 2048).astype(f16))
        m = dict(shared)
        m["xm1"] = xm1
        in_maps.append(m)
    return in_maps, has_bdec


def unpack_output(res_core):
    """[128, 2048] f32 parity-major canvas^T -> [BL, 64, 64]."""
    buf = res_core.reshape(128, 32, 64).transpose(1, 0, 2).reshape(4096, BL)
    out = np.empty((4096, BL), np.float32)
    out[PIX] = buf
    return out.T.reshape(BL, IMG, IMG)


_NC_CACHE = {}


def _get_nc(repeat=None, has_bdec=False):
    key = (repeat, has_bdec)
    if key not in _NC_CACHE:
        _NC_CACHE[key] = build_program(repeat=repeat, has_bdec=has_bdec)
    return _NC_CACHE[key]


def kernel(**inputs):
    import sys
    if "/opt/trn_rl_repo" not in sys.path:
        sys.path.insert(0, "/opt/trn_rl_repo")
    from concourse import bass_utils

    in_maps, has_bdec = host_pack(inputs)
    nc = _get_nc(has_bdec=has_bdec)
    res = bass_utils.run_bass_kernel_spmd(nc, in_maps,
                                          core_ids=list(range(NCORES)))
    outs = [unpack_output(np.asarray(res.results[c]["canvas"]))
            for c in range(NCORES)]
    return np.concatenate(outs, axis=0).astype(
        np.asarray(inputs["x"]).dtype, copy=False)


# revision 8
# speedup vs baseline: 1.3792x; 1.0820x over previous
"""DRAW-model Trainium2 kernel (8 NeuronCores, data-parallel over batch).

Strategy (v2)
-------------
Pure data parallelism: 8 cores x 64 local batch, zero collectives.  All
weights SBUF-resident in fp16.  Activations feature-major ("transposed")
so the big encoder matmul streams N=512-wide moving operands at full PE
rate.  Key structural choices:

  * canvas lives permanently in PSUM (4 banks), accumulated across steps
    by the tensor engine (start=False accumulation);
  * pixels in "parity plane" order; extract_patches folded into the
    encoder weight (host side);
  * FEATURE-SPLIT column tiling: both LSTM matmuls run two concurrent
    64-col PE groups, where group A computes all four gates for features
    0-127 (PSUM partitions 0-63) and group B for features 128-255
    (partitions 64-127).  The LSTM cell math is elementwise per feature,
    so both halves proceed with no cross-partition traffic (v1 needed a
    1.9us SBUF->SBUF DMA partition shift per step);
  * all sigmoids become tanh via sigmoid(x) = 0.5*tanh(0.5x)+0.5 with
    the 0.5 input scales folded into the weights, so every activation
    (tanh, exp, copy) lives in ONE ACT table set -> zero per-step
    ACT_TABLE_LOADs (v1 paid 2x 1.28us per step);
  * x_hat = x - sigmoid(canvas) is tracked as x_hat_alt = 2*x_hat =
    (2x-1) + tanh(-canvas/2), with the compensating 0.5 folded into the
    encoder weight pixel rows;
  * attention softmax reformulated transposed (exp -> ones-matmul column
    sum -> reciprocal -> K=1 broadcast matmul);
  * biases folded in as extra contraction rows.
"""

import numpy as np

STEPS = 10
UNITS = 256
BL = 64          # local batch per core
NCORES = 8
IMG = 64

# ---------------------------------------------------------------- host index math
def _pix_order():
    # new pixel index n = q*1024 + j*32 + i  ->  original pixel (2j+pr)*64 + (2i+pc)
    # with q = pr*2 + pc
    out = np.empty(4096, np.int64)
    n = 0
    for pr in range(2):
        for pc in range(2):
            for j in range(32):
                for i in range(32):
                    out[n] = (2 * j + pr) * 64 + (2 * i + pc)
                    n += 1
    return out


PIX = _pix_order()


def fold_enc_kernel(W):
    """Collapse extract_patches into the weight: each patch feature copies one
    pixel of x_hat, so patches @ W[:9216] == x_hat_flat @ A with
    A[p,:] = sum of W rows whose feature reads pixel p.  [4096, 1024]."""
    A = np.zeros((4096, W.shape[1]), np.float32)
    r_idx = np.arange(32)
    for dy in range(3):
        rows = 2 * r_idx + dy
        rv = r_idx[rows < 64]
        for dx in range(3):
            cols = 2 * r_idx + dx
            cv = r_idx[cols < 64]
            pix = (2 * rv[:, None] + dy) * 64 + (2 * cv[None, :] + dx)
            feat = (rv[:, None] * 32 + cv[None, :]) * 9 + (dy * 3 + dx)
            A[pix.ravel()] += W[feat.ravel()]
    return A


def repack_gate_cols(W):
    """[rows, 1024] with keras layout [i|f|g|o] x 256 features ->
    feature-split layout: cols [h*512 + g*128 + j] = old [g*256 + h*128 + j].
    Also folds the sigmoid-as-tanh input scale 0.5 into the i, f, o gates."""
    out = np.empty_like(W)
    for h in range(2):
        for g in range(4):
            s = 0.5 if g != 2 else 1.0
            out[:, h * 512 + g * 128: h * 512 + (g + 1) * 128] = (
                W[:, g * 256 + h * 128: g * 256 + (h + 1) * 128] * s)
    return out


# ---------------------------------------------------------------- program builder
def build_program(repeat=None, steps=STEPS, has_bdec=False, carry_state=False):
    """Build + compile the per-core Bass program.  Returns nc."""
    import concourse.bacc as bacc
    import concourse.tile as tile
    import concourse.mybir as mybir

    f16 = mybir.dt.float16
    f32 = mybir.dt.float32
    AF = mybir.ActivationFunctionType
    ALU = mybir.AluOpType

    nc = bacc.Bacc("TRN2", target_bir_lowering=False, debug=False,
                   dynamic_dma_scratch_size=2048)

    def din(name, shape, dt):
        return nc.dram_tensor(name, shape, dt, kind="ExternalInput")

    d_encw = din("encw", [128, 32 * 1024], f16)
    d_recw = din("recw", [128, 5 * 1024], f16)
    d_decw = din("decw", [128, 5 * 1024], f16)
    d_wdec = din("wdec", [128, 2 * 4096], f16)
    d_wenc = din("wenc", [128, 32], f16)
    d_wencT = din("wencT", [16, 256], f16)
    d_benc = din("benc", [16, 1], f32)
    d_iden = din("iden", [128, 128], f16)
    d_onescol = din("onescol", [128, 1], f16)
    d_onesst = din("onesst", [128, 64], f16)
    d_onesrow = din("onesrow", [1, 128], f16)
    d_xm1 = din("xm1", [128, 2048], f16)
    if has_bdec:
        d_bdec = din("bdec", [1, 4096], f16)
    d_out = nc.dram_tensor("canvas", [128, 2048], f32, kind="ExternalOutput")

    with tile.TileContext(nc) as tc:
        # ---------------- static SBUF (raw allocs; tracked via shadow memory)
        s_encw = nc.alloc_sbuf_tensor("s_encw", [128, 32, 1024], f16)
        s_recw = nc.alloc_sbuf_tensor("s_recw", [128, 5, 1024], f16)
        s_decw = nc.alloc_sbuf_tensor("s_decw", [128, 5, 1024], f16)
        s_wdec = nc.alloc_sbuf_tensor("s_wdec", [128, 2, 4096], f16)
        s_wenc = nc.alloc_sbuf_tensor("s_wenc", [128, 2, 16], f16)
        s_wencT = nc.alloc_sbuf_tensor("s_wencT", [16, 256], f16)
        s_benc = nc.alloc_sbuf_tensor("s_benc", [16, 1], f32)
        s_iden = nc.alloc_sbuf_tensor("s_iden", [128, 128], f16)
        s_onescol = nc.alloc_sbuf_tensor("s_onescol", [128, 1], f16)
        s_onesst = nc.alloc_sbuf_tensor("s_onesst", [128, 64], f16)
        s_onesrow = nc.alloc_sbuf_tensor("s_onesrow", [1, 128], f16)
        s_xm1 = nc.alloc_sbuf_tensor("s_xm1", [128, 4, 8, 64], f16)
        s_xhat = nc.alloc_sbuf_tensor("s_xhat", [128, 4, 8, 64], f16)
        s_hencT = nc.alloc_sbuf_tensor("s_hencT", [128, 2, 64], f16)
        s_hdecT = nc.alloc_sbuf_tensor("s_hdecT", [128, 2, 64], f16)
        s_zattnT = nc.alloc_sbuf_tensor("s_zattnT", [128, 2, 64], f16)
        s_cenc = nc.alloc_sbuf_tensor("s_cenc", [128, 128], f32)
        s_cdec = nc.alloc_sbuf_tensor("s_cdec", [128, 128], f32)
        if has_bdec:
            s_bdec = nc.alloc_sbuf_tensor("s_bdec", [1, 4096], f16)

        # ---------------- load weights / constants (outside any repeat loop)
        encw_d_ap = d_encw.ap().rearrange("p (t n) -> p t n", n=1024)
        nc.sync.dma_start(out=s_xm1[:, :, :, :], in_=d_xm1.ap())
        nc.sync.dma_start(out=s_recw[:, :, :], in_=d_recw.ap())
        nc.sync.dma_start(out=s_onesst[:, :], in_=d_onesst.ap())
        for g in range(8):
            sl = slice(g * 2, (g + 1) * 2)
            nc.sync.dma_start(out=s_encw[:, sl, :], in_=encw_d_ap[:, sl, :])
        nc.sync.dma_start(out=s_decw[:, :, :], in_=d_decw.ap())
        for dst, src in [
            (s_wenc[:, :, :], d_wenc.ap()), (s_wencT[:, :], d_wencT.ap()),
            (s_benc[:, :], d_benc.ap()), (s_iden[:, :], d_iden.ap()),
            (s_onescol[:, :], d_onescol.ap()),
            (s_onesrow[:, :], d_onesrow.ap()),
        ]:
            nc.sync.dma_start(out=dst, in_=src)
        for g in range(8, 16):
            sl = slice(g * 2, (g + 1) * 2)
            nc.sync.dma_start(out=s_encw[:, sl, :], in_=encw_d_ap[:, sl, :])
        nc.sync.dma_start(out=s_wdec[:, :, :], in_=d_wdec.ap())
        if has_bdec:
            nc.sync.dma_start(out=s_bdec[:, :], in_=d_bdec.ap())

        # ---------------- pools
        import contextlib
        ctx = contextlib.ExitStack()
        work = ctx.enter_context(tc.tile_pool(name="work", bufs=2))
        p_z = ctx.enter_context(tc.tile_pool(name="p_z", bufs=1, space="PSUM"))
        p_sm = ctx.enter_context(tc.tile_pool(name="p_sm", bufs=3, space="PSUM"))
        p_cv = ctx.enter_context(tc.tile_pool(name="p_cv", bufs=1, space="PSUM"))

        def gates(psZ, c_s, hT_dst):
            """LSTM gate math on the feature-split layout.  psZ: [128, 512]
            PSUM AP with per-partition-half columns [i|f|g|o] x 128 features
            (i, f, o pre-scaled by 0.5 in the weights).  c_s: [128, 128] f32
            state, updated in place.  hT_dst: [128, 128] view of the
            transposed-h fp16 destination."""
            t = work.tile([128, 512], f32, tag="tz")
            nc.scalar.activation(t[:, :], psZ[:, 0:512], AF.Tanh)
            sf = work.tile([128, 128], f32, tag="sf")
            nc.vector.tensor_scalar(sf[:, :], t[:, 128:256], 0.5, 0.5,
                                    ALU.mult, ALU.add)
            nc.vector.tensor_mul(c_s[:, :], sf[:, :], c_s[:, :])
            si = work.tile([128, 128], f32, tag="si")
            nc.vector.tensor_scalar(si[:, :], t[:, 0:128], 0.5, 0.5,
                                    ALU.mult, ALU.add)
            u = work.tile([128, 128], f32, tag="u")
            nc.vector.tensor_mul(u[:, :], si[:, :], t[:, 256:384])
            nc.vector.tensor_add(c_s[:, :], c_s[:, :], u[:, :])
            tc_ = work.tile([128, 128], f32, tag="tc")
            nc.scalar.activation(tc_[:, :], c_s[:, :], AF.Tanh)
            so = work.tile([128, 128], f32, tag="so")
            nc.vector.tensor_scalar(so[:, :], t[:, 384:512], 0.5, 0.5,
                                    ALU.mult, ALU.add)
            h = work.tile([128, 128], f16, tag="h")
            nc.vector.tensor_mul(h[:, :], so[:, :], tc_[:, :])
            pt = p_sm.tile([128, 128], f16, tag="sm")
            nc.tensor.transpose(pt[:, :], h[:, :], s_iden[:, :])
            nc.vector.tensor_scalar_mul(hT_dst, pt[:, :], 1.0)

        def body():
            if not carry_state:
                nc.vector.memset(s_cenc[:, :], 0.0)
                nc.vector.memset(s_cdec[:, :], 0.0)
            canvas = p_cv.tile([128, 32, 64], f32, tag="canvas")

            for t in range(steps):
                # ---- encoder matmul; group A -> psZ[0:64] (features 0-127),
                # group B -> psZ[64:128] (features 128-255)
                psZ = p_z.tile([128, 512], f32, tag="z")
                stat = []   # (stationary AP, rhs buffer handle, rhs tile idx)
                if t > 0:
                    stat.append((s_hencT[:, 0, :], s_recw, 0))
                    stat.append((s_hencT[:, 1, :], s_recw, 1))
                    stat.append((s_hdecT[:, 0, :], s_recw, 2))
                    stat.append((s_hdecT[:, 1, :], s_recw, 3))
                stat.append((s_onesst[:, :], s_recw, 4))
                xh = s_xm1 if t == 0 else s_xhat
                for q in range(4):
                    for m in range(8):
                        stat.append((xh[:, q, m, :], s_encw, q * 8 + m))
                last = len(stat) - 1
                for j, (st, buf, jj) in enumerate(stat):
                    nc.tensor.matmul(
                        psZ[0:64, :], st, buf[:, jj, 0:512],
                        start=(j == 0), stop=(j == last),
                        tile_position=(0, 0), skip_group_check=True)
                    nc.tensor.matmul(
                        psZ[64:128, :], st, buf[:, jj, 512:1024],
                        start=(j == 0), stop=(j == last),
                        tile_position=(0, 64), skip_group_check=True)
                gates(psZ[:, :], s_cenc, s_hencT[:, :, :])

                # ---- attention (transposed, softmax via ones-matmul)
                ps_log = p_sm.tile([16, 64], f32, tag="sm")
                nc.tensor.matmul(ps_log[0:10, :], s_wenc[:, 0, 0:10],
                                 s_hencT[:, 0, :], start=True, stop=False,
                                 skip_group_check=True)
                nc.tensor.matmul(ps_log[0:10, :], s_wenc[:, 1, 0:10],
                                 s_hencT[:, 1, :], start=False, stop=True,
                                 skip_group_check=True)
                expT = work.tile([16, 64], f16, tag="expT")
                nc.scalar.activation(expT[0:10, :], ps_log[0:10, :], AF.Exp,
                                     bias=s_benc[0:10, 0:1])
                ps_cs = p_sm.tile([16, 64], f32, tag="sm")
                nc.tensor.matmul(ps_cs[0:1, :], s_onescol[0:10, 0:1],
                                 expT[0:10, :], start=True, stop=True,
                                 skip_group_check=True)
                rec32 = work.tile([1, 64], f32, tag="rec32")
                nc.vector.reciprocal(rec32[:, :], ps_cs[0:1, :])
                rec16 = work.tile([1, 64], f16, tag="rec16")
                nc.vector.tensor_scalar_mul(rec16[:, :], rec32[:, :], 1.0)
                ps_bc = p_sm.tile([128, 64], f32, tag="sm")
                nc.tensor.matmul(ps_bc[:, :], s_onesrow[0:1, 0:128],
                                 rec16[0:1, :], start=True, stop=True,
                                 skip_group_check=True)
                bc = work.tile([128, 64], f32, tag="bcs")
                nc.scalar.activation(bc[:, :], ps_bc[:, :], AF.Copy)
                for k in range(2):
                    ps_zat = p_sm.tile([128, 64], f32, tag="sm")
                    nc.tensor.matmul(ps_zat[:, :],
                                     s_wencT[0:10, k * 128:(k + 1) * 128],
                                     expT[0:10, :], start=True, stop=True,
                                     skip_group_check=True)
                    nc.vector.tensor_mul(s_zattnT[:, k, :], ps_zat[:, :],
                                         bc[:, :])

                # ---- decoder LSTM matmul (feature-split col-tiled like enc)
                psZ2 = p_z.tile([128, 512], f32, tag="z")
                dstat = []
                if t > 0:
                    dstat += [(s_hdecT[:, 0, :], 2), (s_hdecT[:, 1, :], 3)]
                dstat.append((s_onesst[:, :], 4))
                dstat += [(s_zattnT[:, 0, :], 0), (s_zattnT[:, 1, :], 1)]
                dlast = len(dstat) - 1
                for j, (st, jj) in enumerate(dstat):
                    nc.tensor.matmul(psZ2[0:64, :], st, s_decw[:, jj, 0:512],
                                     start=(j == 0), stop=(j == dlast),
                                     tile_position=(0, 0),
                                     skip_group_check=True)
                    nc.tensor.matmul(psZ2[64:128, :], st,
                                     s_decw[:, jj, 512:1024],
                                     start=(j == 0), stop=(j == dlast),
                                     tile_position=(0, 64),
                                     skip_group_check=True)
                gates(psZ2[:, :], s_cdec, s_hdecT[:, :, :])

                # ---- canvas += W_dec^T @ h_dec  (PSUM-resident accumulation)
                # plane-major so bank q completes early and the next step's
                # tanh(plane q) / enc chunk matmuls can overlap banks q+1..3
                for q in range(4):
                    for k in range(2):
                        for m in range(8 * q, 8 * (q + 1)):
                            nc.tensor.matmul(
                                canvas[:, m, :],
                                s_wdec[:, k, m * 128:(m + 1) * 128],
                                s_hdecT[:, k, :],
                                start=(t == 0 and k == 0 and m % 8 == 0),
                                stop=(t == steps - 1 and k == 1
                                      and not has_bdec),
                                skip_group_check=True)
                if has_bdec:
                    for m in range(32):
                        nc.tensor.matmul(
                            canvas[:, m, :],
                            s_bdec[0:1, m * 128:(m + 1) * 128],
                            s_onesrow[0:1, 0:64],
                            start=False,
                            stop=(t == steps - 1 and m == 31),
                            skip_group_check=True)

                # ---- x_hat_alt for the next step: (2x-1) + tanh(-canvas/2)
                if t < steps - 1:
                    for q in range(4):
                        xq = s_xhat[:, q, :, :]
                        nc.scalar.activation(xq, canvas[:, 8 * q:8 * (q + 1), :],
                                             AF.Tanh, scale=-0.5)
                        nc.vector.tensor_add(xq, xq, s_xm1[:, q, :, :])

            # evacuate canvas PSUM -> SBUF -> DRAM per bank (ACT/DVE alternate)
            for q in range(4):
                cv = work.tile([128, 512], f32, tag="cvout")
                if q % 2 == 0:
                    nc.scalar.activation(cv[:, :],
                                         canvas[:, 8 * q:8 * (q + 1), :],
                                         AF.Copy)
                else:
                    nc.vector.tensor_scalar_mul(
                        cv[:, :], canvas[:, 8 * q:8 * (q + 1), :], 1.0)
                nc.sync.dma_start(
                    out=d_out.ap()[:, q * 512:(q + 1) * 512], in_=cv[:, :])

        if carry_state:
            nc.vector.memset(s_cenc[:, :], 0.0)
            nc.vector.memset(s_cdec[:, :], 0.0)
        if repeat:
            with tc.For_i(0, repeat, 1):
                body()
        else:
            body()
        ctx.close()

    nc.compile()
    return nc


# ---------------------------------------------------------------- host packing
def host_pack(inputs):
    """Preprocess full inputs -> (per-core input maps, has_bdec)."""
    f16 = np.float16
    ek = np.asarray(inputs["enc_kernel"], np.float32)
    # pixel rows scaled by 0.5: the kernel feeds x_hat_alt = 2*x_hat
    A = repack_gate_cols(fold_enc_kernel(ek[:9216])[PIX] * 0.5)
    hdf = repack_gate_cols(ek[9216:9472] + ek[9472:9728])
    enc_rec = repack_gate_cols(np.asarray(inputs["enc_rec"], np.float32))
    enc_bias = np.asarray(inputs["enc_bias"], np.float32)
    dec_k = repack_gate_cols(np.asarray(inputs["dec_kernel"], np.float32))
    dec_rec = repack_gate_cols(np.asarray(inputs["dec_rec"], np.float32))
    dec_bias = np.asarray(inputs["dec_bias"], np.float32)
    W_enc = np.asarray(inputs["W_enc"], np.float32)
    b_enc = np.asarray(inputs["b_enc"], np.float32)
    W_dec = np.asarray(inputs["W_dec"], np.float32)
    b_dec = np.asarray(inputs["b_dec"], np.float32)

    encw = A.reshape(32, 128, 1024).transpose(1, 0, 2).reshape(128, -1)

    def brow(bias1024):
        t = np.zeros((128, 1024), np.float32)
        t[0] = bias1024
        return t

    eb = repack_gate_cols(enc_bias[None, :])[0]
    db = repack_gate_cols(dec_bias[None, :])[0]
    recw = np.stack([enc_rec[0:128], enc_rec[128:256], hdf[0:128], hdf[128:256],
                     brow(eb)]).transpose(1, 0, 2).reshape(128, -1)
    decw = np.stack([dec_k[0:128], dec_k[128:256], dec_rec[0:128],
                     dec_rec[128:256], brow(db)]
                    ).transpose(1, 0, 2).reshape(128, -1)
    wdec = W_dec[:, PIX].reshape(2, 128, 4096).transpose(1, 0, 2).reshape(128, -1)
    wenc = np.zeros((128, 2, 16), np.float32)
    wenc[:, 0, 0:10] = W_enc[0:128]
    wenc[:, 1, 0:10] = W_enc[128:256]
    wencT = np.zeros((16, 256), np.float32)
    wencT[0:10] = W_enc.T
    benc = np.zeros((16, 1), np.float32)
    benc[0:10, 0] = b_enc
    onesst = np.zeros((128, 64), np.float32)
    onesst[0] = 1.0

    shared = {
        "encw": encw.astype(f16), "recw": recw.astype(f16),
        "decw": decw.astype(f16), "wdec": wdec.astype(f16),
        "wenc": wenc.reshape(128, 32).astype(f16),
        "wencT": wencT.astype(f16), "benc": benc,
        "iden": np.eye(128, dtype=f16),
        "onescol": np.ones((128, 1), f16),
        "onesst": onesst.astype(f16),
        "onesrow": np.ones((1, 128), f16),
    }
    has_bdec = bool(np.any(b_dec))
    if has_bdec:
        shared["bdec"] = b_dec[PIX].reshape(1, 4096).astype(f16)

    x = np.asarray(inputs["x"], np.float32)
    B = x.shape[0]
    assert B == NCORES * BL, f"expected batch {NCORES * BL}, got {B}"
    in_maps = []
    for c in range(NCORES):
        xc = 2.0 * x[c * BL:(c + 1) * BL].reshape(BL, 4096).T[PIX] - 1.0
        xm1 = (xc.reshape(4, 8, 128, BL).transpose(2, 0, 1, 3)
               .reshape(128, 2048).astype(f16))
        m = dict(shared)
        m["xm1"] = xm1
        in_maps.append(m)
    return in_maps, has_bdec


def unpack_output(res_core):
    """[128, 2048] f32 parity-major canvas^T -> [BL, 64, 64]."""
    buf = res_core.reshape(128, 32, 64).transpose(1, 0, 2).reshape(4096, BL)
    out = np.empty((4096, BL), np.float32)
    out[PIX] = buf
    return out.T.reshape(BL, IMG, IMG)


_NC_CACHE = {}


def _get_nc(repeat=None, has_bdec=False):
    key = (repeat, has_bdec)
    if key not in _NC_CACHE:
        _NC_CACHE[key] = build_program(repeat=repeat, has_bdec=has_bdec)
    return _NC_CACHE[key]


def kernel(**inputs):
    import sys
    if "/opt/trn_rl_repo" not in sys.path:
        sys.path.insert(0, "/opt/trn_rl_repo")
    from concourse import bass_utils

    in_maps, has_bdec = host_pack(inputs)
    nc = _get_nc(has_bdec=has_bdec)
    res = bass_utils.run_bass_kernel_spmd(nc, in_maps,
                                          core_ids=list(range(NCORES)))
    outs = [unpack_output(np.asarray(res.results[c]["canvas"]))
            for c in range(NCORES)]
    return np.concatenate(outs, axis=0).astype(
        np.asarray(inputs["x"]).dtype, copy=False)
